# revision 13
# baseline (speedup 1.0000x reference)
"""Trainium2 Bass kernel for nn_CMB_H_OMBH2 (MLP -> natural cubic spline -> grid eval).

Strategy (v3):
  - The eval grid x = sqrt(i^2+j^2) is mirror-symmetric: only the 129x129
    block is unique (25% of points).  Cores compute the unique block
    (2112 points each, data-parallel); the host mirrors rows/cols back.
  - x <= 181.02 while knots[10] = 200, so only spline intervals 0..9 are
    ever active.  The clamped truncated-power basis needs just 16 knots:
        val(x) = a0 + sum_{j<16} [ w1_j*u_j + w2_j*u_j^2 + w3_j*u_j^3 ],
        u_j = clip(x - kn_j, 0, h_j)
    exact for x in [kn_0, kn_16] by spline-coefficient continuity.
  - The tridiagonal solve is truncated to the leading 32x32 system (the
    inverse of this diagonally dominant tridiagonal decays geometrically)
    and solved with 3 Newton-Schulz iterations on the PE.
  - The coefficient pipeline collapses to  W48 = GxT^T@(T32^T@y) + Dd^T@y
    where GxT/T32/Dd depend only on knots and build in parallel with the MLP.
  - Inputs arrive in 3 packed DMAs (host does pure layout marshalling);
    MLP weights ship as f32r; eval = 16 f32r matmuls [48]x[128ch x 264pts];
    bias-fused PSUM->SBUF copies cast to bf16; outputs stream in 6 DMAs
    split across the SP and Pool DGE paths.
  - Preconditions (exact knots pattern, grid symmetry, range) are verified
    on the host; any mismatch falls back to an exact numpy path.
"""
import sys
import numpy as np

sys.path.insert(0, "/opt/trn_rl_repo")

N_CORES = 8
NK = 16          # knots in eval basis
NT = 32          # truncated interior tridiagonal system
NI = 34          # y rows needed (interior knots 1..32 -> y[0..33])
GRP = 8          # point groups per core
P = 264          # points per group
PTS = GRP * P    # 2112 points per core
UNIQ = 129 * 129 # unique grid points
THETA_LO = (50.0, 0.0075)
THETA_SCALE = (40.0, 0.0492)

P1_COLS = 366    # thetaT | kn windows | w0 | lo isc | b0 b1 b2
P2_COLS = 268    # b3 | knp | xrep
P3_COLS = 328    # w1 | w2 | w3  (f32r)

_CACHE = {}


def _build_program():
    import concourse.bacc as bacc
    import concourse.tile as tile
    import concourse.mybir as mybir

    dt = mybir.dt
    Alu = mybir.AluOpType
    Act = mybir.ActivationFunctionType

    f32 = dt.float32
    f32r = dt.float32r
    bf16 = dt.bfloat16

    nc = bacc.Bacc("TRN2", target_bir_lowering=False, debug=False,
                   num_devices=N_CORES)

    p1_d = nc.dram_tensor("p1", [100, P1_COLS], f32, kind="ExternalInput").ap()
    p2_d = nc.dram_tensor("p2", [128, P2_COLS], f32, kind="ExternalInput").ap()
    p3_d = nc.dram_tensor("p3", [100, P3_COLS], f32, kind="ExternalInput").ap()
    out_d = nc.dram_tensor("out", [256, PTS], bf16, kind="ExternalOutput").ap()

    with tile.TileContext(nc) as tc:
        with (
            tc.tile_pool(name="const", bufs=1) as cpool,
            tc.tile_pool(name="newton", bufs=2) as npool,
            tc.tile_pool(name="mlpps", bufs=2, space="PSUM") as mpsum,
            tc.tile_pool(name="smps", bufs=2, space="PSUM") as spsum,
            tc.tile_pool(name="evps", bufs=4, space="PSUM") as epsum,
        ):
            # ============ packed input DMAs (sync) ============
            p1 = cpool.tile([100, P1_COLS], f32)
            nc.sync.dma_start(p1[:], p1_d[:])
            p2 = cpool.tile([128, P2_COLS], f32)
            nc.sync.dma_start(p2[:], p2_d[:])
            p3 = cpool.tile([100, P3_COLS], f32r)
            nc.sync.dma_start(p3[:], p3_d[:].bitcast(f32r))

            thetaT = p1[0:2, 0:256]
            knc = p1[0:35, 256:257]
            kn1c = p1[0:35, 257:258]
            kn2c = p1[0:35, 258:259]
            knm1c = p1[0:35, 260:261]
            w0sb = p1[0:2, 261:361]
            lo_c = p1[0:2, 361:362]
            isc_c = p1[0:2, 362:363]
            b0c = p1[0:100, 363:364]
            b1c = p1[0:100, 364:365]
            b2c = p1[0:100, 365:366]
            b3c = p2[0:128, 0:1]
            knp = p2[:, 1:3]
            xrep = p2[:, 4:268]
            w1r = p3[0:100, 0:100]
            w2r = p3[0:100, 100:200]
            w3r = p3[0:100, 200:328]

            # ============ knot-derived columns (DVE) ============
            # cmul blocks (dep-free memsets run first)
            cm3 = cpool.tile([35, 3], f32)   # (6, 6, -1/6)
            nc.vector.memset(cm3[:, 0:2], 6.0)
            nc.vector.memset(cm3[:, 2:3], float(-1.0 / 6.0))
            cm2 = cpool.tile([35, 2], f32)   # (-1/6, -1/3)
            nc.vector.memset(cm2[:, 0:1], float(-1.0 / 6.0))
            nc.vector.memset(cm2[:, 1:2], float(-1.0 / 3.0))
            cm2b = cpool.tile([35, 2], f32)  # (1/6, -1)
            nc.vector.memset(cm2b[:, 0:1], float(1.0 / 6.0))
            nc.vector.memset(cm2b[:, 1:2], -1.0)
            halfc = cpool.tile([32, 1], f32)
            nc.vector.memset(halfc[:], 0.5)
            twoc = cpool.tile([32, 1], f32)
            nc.vector.memset(twoc[:], 2.0)
            onec = cpool.tile([128, 1], f32)
            nc.vector.memset(onec[:], 1.0)

            cols = cpool.tile([35, 16], f32)
            hm1_c = cols[:, 0:1]    # h_{k-1} (rows >= 1)
            h_c = cols[:, 1:2]      # h_k
            h1_c = cols[:, 2:3]     # h_{k+1}
            rhm1_c = cols[:, 3:4]   # 1/h_{k-1}
            rh_c = cols[:, 4:5]     # 1/h_k
            rh1_c = cols[:, 5:6]    # 1/h_{k+1}
            srhm6_c = cols[:, 6:7]  # 6/h_{k-1}
            srh6_c = cols[:, 7:8]   # 6/h_k
            nrh61_c = cols[:, 8:9]  # -1/(6 h_{k+1})
            hn6_c = cols[:, 9:10]   # -h_k/6
            l2_c = cols[:, 10:11]   # -h_{k+1}/3
            rh6_c = cols[:, 11:12]  # 1/(6 h_k)
            nrh_c = cols[:, 12:13]  # -1/h_k
            tsum_c = cols[:, 13:14]
            nsrh_c = cols[:, 14:15]  # -6(1/h_{k-1} + 1/h_k)
            dg_c = cols[:, 15:16]    # 2(h_k + h_{k+1})
            rd_c = cpool.tile([35, 1], f32)

            nc.vector.tensor_tensor(cols[:, 1:3], p1[0:35, 257:259],
                                    p1[0:35, 256:258], Alu.subtract)  # h, h1
            nc.vector.tensor_tensor(hm1_c, knc, knm1c, Alu.subtract)
            nc.vector.reciprocal(cols[:, 3:6], cols[:, 0:3])
            # tn (theta norm) early on DVE
            tn = cpool.tile([2, 256], f32r)
            nc.vector.tensor_scalar(tn[:], thetaT, lo_c, isc_c,
                                    Alu.subtract, Alu.mult)
            nc.vector.tensor_tensor(cols[:, 6:9], cols[:, 3:6], cm3[:],
                                    Alu.mult)  # srhm6, srh6, nrh61
            nc.vector.tensor_tensor(cols[:, 9:11], cols[:, 1:3], cm2[:],
                                    Alu.mult)  # hn6, l2
            nc.vector.tensor_tensor(cols[:, 11:13],
                                    rh_c.broadcast_to([35, 2]), cm2b[:],
                                    Alu.mult)  # rh6, nrh
            nc.vector.tensor_tensor(tsum_c, rhm1_c, rh_c, Alu.add)
            nc.vector.tensor_scalar_mul(nsrh_c, tsum_c, -6.0)
            nc.vector.tensor_tensor(dg_c, h_c, h1_c, Alu.add)
            nc.vector.tensor_scalar_mul(dg_c, dg_c, 2.0)
            nc.vector.reciprocal(rd_c[:], dg_c)

            # ============ selector matrices (Pool) ============
            def sel(out_ap, col_ap, base, n_free):
                nc.gpsimd.affine_select(out_ap, col_ap.broadcast_to(
                    [out_ap.shape[0], n_free]),
                    pattern=[[-1, n_free]], base=base, channel_multiplier=1,
                    compare_op=Alu.is_equal, fill=0.0)

            # critical-first: A32, i2, X0, ident, T32T, SH_S
            hp = tc.high_priority
            a32 = cpool.tile([NT, NT], f32)
            a_u = cpool.tile([NT, NT], f32)
            a_l = cpool.tile([NT, NT], f32)
            i2 = cpool.tile([NT, NT], f32)
            x0 = npool.tile([NT, NT], f32, tag="xn")
            ident = cpool.tile([128, 128], f32)
            with hp():
                sel(a32[:], dg_c[0:NT, :], 0, NT)
                sel(a_u[:], h1_c[0:NT, :], 1, NT)
                sel(a_l[:], h_c[0:NT, :], -1, NT)
                sel(i2[:], twoc[0:NT, :], 0, NT)
                sel(x0[:], rd_c[0:NT, :], 0, NT)
                nc.vector.tensor_tensor(a32[:], a32[:], a_u[:], Alu.add)
                nc.vector.tensor_tensor(a32[:], a32[:], a_l[:], Alu.add)
            sel(ident[:], onec[:], 0, 128)
            t32raw = cpool.tile([NI, NT], f32)
            t_t1 = cpool.tile([NI, NT], f32)
            t_t2 = cpool.tile([NI, NT], f32)
            sel(t32raw[:], srh6_c[0:NI, :], 0, NT)
            sel(t_t1[:], nsrh_c[0:NI, :], -1, NT)
            sel(t_t2[:], srhm6_c[0:NI, :], -2, NT)
            sh_s = cpool.tile([NT, NK], f32)
            sel(sh_s[:], halfc[0:NT, :], 1, NK)

            # late selectors (needed ~7.5us): SH_L, SH_C, Dd
            sh_l = cpool.tile([NT, NK], f32)
            sh_t = cpool.tile([NT, NK], f32)
            sel(sh_l[:], l2_c[0:NT, :], 1, NK)
            sel(sh_t[:], hn6_c[0:NT, :], 0, NK)
            sh_c = cpool.tile([NT, NK], f32)
            sh_t2 = cpool.tile([NT, NK], f32)
            sel(sh_c[:], rh6_c[0:NT, :], 0, NK)
            sel(sh_t2[:], nrh61_c[0:NT, :], 1, NK)
            dd_raw = cpool.tile([NI, 48], f32)
            nc.gpsimd.memset(dd_raw[:, NK:48], 0.0)
            d_t1 = cpool.tile([NI, NK], f32)
            sel(dd_raw[:, 0:NK], nrh_c[0:NI, :], 0, NK)
            sel(d_t1[:], rhm1_c[0:NI, :], -1, NK)

            # ============ f32r weight copy for layer 0 (Act) ============
            w0r = cpool.tile([2, 100], f32r)
            h0p = mpsum.tile([100, 256], f32, tag="mp")
            h0t = cpool.tile([100, 256], f32r)
            with hp():
                nc.scalar.copy(w0r[:], w0sb)
                nc.tensor.matmul(h0p[:], w0r[:], tn[:], start=True, stop=True)
                nc.scalar.activation(h0t[:], h0p[:], Act.Relu, bias=b0c)

            t32t_f = cpool.tile([NI, NT], f32r)
            x_cur = x0
            for it in range(3):
                eps = spsum.tile([NT, NT], f32, tag="sp")
                nc.tensor.matmul(eps[:], a32[:], x_cur[:], start=True, stop=True)
                y_n = npool.tile([NT, NT], f32, tag="yn")
                nc.vector.scalar_tensor_tensor(y_n[:], eps[:], -1.0, i2[:],
                                               Alu.mult, Alu.add)
                xps = spsum.tile([NT, NT], f32, tag="sp")
                nc.tensor.matmul(xps[:], x_cur[:], y_n[:], start=True, stop=True)
                x_new = npool.tile([NT, NT], f32, tag="xn")
                nc.vector.tensor_copy(x_new[:], xps[:])
                x_cur = x_new
                if it == 1:
                    # T32T adds slot in here (selectors ready by now)
                    nc.vector.tensor_tensor(t32raw[:], t32raw[:], t_t1[:],
                                            Alu.add)
                    nc.vector.tensor_tensor(t32t_f[:], t32raw[:], t_t2[:],
                                            Alu.add)
                if it == 0:
                    h1p = mpsum.tile([100, 256], f32, tag="mp")
                    h1t = cpool.tile([100, 256], f32r)
                    with hp():
                        nc.tensor.matmul(h1p[:], w1r, h0t[:], start=True,
                                         stop=True)
                        nc.scalar.activation(h1t[:], h1p[:], Act.Relu, bias=b1c)
                elif it == 1:
                    h2p = mpsum.tile([100, 256], f32, tag="mp")
                    h2t = cpool.tile([100, 256], f32r)
                    with hp():
                        nc.tensor.matmul(h2p[:], w2r, h1t[:], start=True,
                                         stop=True)
                        nc.scalar.activation(h2t[:], h2p[:], Act.Relu, bias=b2c)
                elif it == 2:
                    h3p = mpsum.tile([128, 256], f32, tag="mp")
                    outT = cpool.tile([128, 256], f32)
                    with hp():
                        nc.tensor.matmul(h3p[:], w3r, h2t[:], start=True,
                                         stop=True)
                        nc.scalar.activation(outT[:], h3p[:], Act.Identity,
                                             bias=b3c)
            x32 = x_cur  # [32, 32] ~A32^{-1}

            # ============ y_t via transposes ============
            outT3 = outT[:].rearrange("m (b t) -> m t b", t=2)
            y_t = cpool.tile([NI, 256], f32r)
            tp0 = spsum.tile([NI, 128], f32, tag="sp")
            tp1 = spsum.tile([NI, 128], f32, tag="sp")
            with hp():
                nc.tensor.transpose(tp0[:], outT3[:, 0, 0:NI], ident[:])
                nc.scalar.copy(y_t[:, 0:128], tp0[:])
                nc.tensor.transpose(tp1[:], outT3[:, 1, 0:NI], ident[:])
                nc.vector.tensor_copy(y_t[:, 128:256], tp1[:])

            # late DVE adds: Dd, SH
            t32t = t32t_f
            dd = cpool.tile([NI, 48], f32r)
            nc.vector.tensor_tensor(dd_raw[:, 0:NK], dd_raw[:, 0:NK], d_t1[:],
                                    Alu.add)
            nc.vector.tensor_copy(dd[:], dd_raw[:])
            nc.vector.tensor_tensor(sh_l[:], sh_l[:], sh_t[:], Alu.add)
            nc.vector.tensor_tensor(sh_c[:], sh_c[:], sh_t2[:], Alu.add)

            # ============ rhs32 = T32 @ y ============
            rp = spsum.tile([NT, 256], f32, tag="sp")
            rhs32 = cpool.tile([NT, 256], f32r)
            with hp():
                nc.tensor.matmul(rp[:], t32t[:], y_t[:], start=True, stop=True)
                nc.scalar.copy(rhs32[:], rp[:])

            # ============ GxT = X32 @ [SH_L SH_S SH_C] ============
            gxp = spsum.tile([NT, 48], f32, tag="sp")
            nc.tensor.matmul(gxp[:, 0:NK], x32[:], sh_l[:], start=True, stop=True)
            nc.tensor.matmul(gxp[:, NK:2 * NK], x32[:], sh_s[:], start=True, stop=True)
            nc.tensor.matmul(gxp[:, 2 * NK:3 * NK], x32[:], sh_c[:], start=True, stop=True)
            gxt = cpool.tile([NT, 48], f32r)
            nc.vector.tensor_copy(gxt[:], gxp[:])

            # ============ W48 ============
            wp = spsum.tile([48, 256], f32, tag="sp")
            w48 = cpool.tile([48, 256], f32r)
            with hp():
                nc.tensor.matmul(wp[:], gxt[:], rhs32[:], start=True, stop=False)
                nc.tensor.matmul(wp[:], dd[:], y_t[:], start=False, stop=True)
                nc.scalar.copy(w48[:, 0:128], wp[:, 0:128])
                nc.vector.tensor_copy(w48[:, 128:256], wp[:, 128:256])

            # ===== basis mega tile (emitted late = low priority; runs in =====
            # ===== engine idle holes well before eval needs ball)        =====
            caps128 = cpool.tile([128, 1], f32)
            nc.vector.tensor_tensor(caps128[:], knp[:, 1:2], knp[:, 0:1],
                                    Alu.subtract)
            mega = cpool.tile([128, 3 * P], f32r)
            nc.vector.tensor_scalar(mega[:, 0:P], xrep, knp[:, 0:1],
                                    caps128[:], Alu.subtract, Alu.min)
            nc.gpsimd.tensor_scalar(mega[:, 0:P], mega[:, 0:P], 0.0,
                                    None, Alu.max)
            nc.gpsimd.tensor_tensor(mega[:, P:2 * P], mega[:, 0:P],
                                    mega[:, 0:P], Alu.mult)
            nc.gpsimd.tensor_tensor(mega[:, 2 * P:3 * P], mega[:, P:2 * P],
                                    mega[:, 0:P], Alu.mult)
            ball = cpool.tile([48, PTS], f32r)
            for c in range(3):
                nc.sync.dma_start(ball[NK * c:NK * (c + 1), :],
                                  mega[:, P * c:P * (c + 1)])

            # ============ eval ============
            obuf0 = cpool.tile([128, PTS], bf16)
            obuf1 = cpool.tile([128, PTS], bf16)
            a0c0 = outT[:, 0:1]
            a0c1 = outT[:, 1:2]
            for g in range(GRP):
                cs = slice(P * g, P * (g + 1))
                vp0 = epsum.tile([128, P], f32, tag="ev")
                nc.tensor.matmul(vp0[:], w48[:, 0:128], ball[:, cs],
                                 start=True, stop=True)
                nc.scalar.activation(obuf0[:, cs], vp0[:], Act.Identity, bias=a0c0)
                vp1 = epsum.tile([128, P], f32, tag="ev")
                nc.tensor.matmul(vp1[:], w48[:, 128:256], ball[:, cs],
                                 start=True, stop=True)
                nc.vector.tensor_scalar(obuf1[:, cs], vp1[:], a0c1, None, Alu.add)
                if g % 2 == 1:
                    lo, hi = P * (g - 1), P * (g + 1)
                    nc.sync.dma_start(out_d[0:128, lo:hi], obuf0[:, lo:hi])
                    nc.gpsimd.dma_start(out_d[128:256, lo:hi], obuf1[:, lo:hi])
    nc.compile()
    return nc


def _expected_knots():
    return (2.0 * np.arange(128, dtype=np.float32) ** 2).astype(np.float32)


def _fast_path_ok(inputs):
    try:
        kn = np.asarray(inputs["knots"], dtype=np.float32)
        grid = np.asarray(inputs["grid"], dtype=np.float32)
        if kn.shape != (128,) or grid.shape != (256, 256):
            return False
        if not np.array_equal(kn, _expected_knots()):
            return False
        if grid.min() < 0.0 or grid.max() >= float(kn[NK]):
            return False
        blk = grid[:129, :129]
        rec = np.empty((256, 256), np.float32)
        rec[:129, :129] = blk
        rec[129:, :129] = blk[127:0:-1, :]
        rec[:, 129:] = rec[:, 127:0:-1]
        return np.array_equal(rec, grid)
    except Exception:
        return False


def _pack_inputs(inputs):
    """Pure layout marshalling of the full inputs into packed arrays."""
    f = np.float32
    kn = np.asarray(inputs["knots"], f)
    p1 = np.zeros((100, P1_COLS), f)
    p1[0:2, 0:256] = np.asarray(inputs["theta"], f).T
    for c in range(3):
        p1[0:35, 256 + c] = kn[c:c + 35]
    p1[0, 260] = -1.0
    p1[1:35, 260] = kn[0:34]
    p1[0:2, 261:361] = np.asarray(inputs["W0"], f)
    p1[0:2, 361] = np.asarray(THETA_LO, f)
    p1[0:2, 362] = (1.0 / np.asarray(THETA_SCALE, f)).astype(f)
    p1[:, 363] = np.asarray(inputs["b0"], f)
    p1[:, 364] = np.asarray(inputs["b1"], f)
    p1[:, 365] = np.asarray(inputs["b2"], f)

    p2 = np.zeros((128, P2_COLS), f)
    p2[0:128, 0] = np.asarray(inputs["b3"], f)
    jj = np.arange(128) // 8
    p2[:, 1] = kn[jj]
    p2[:, 2] = kn[jj + 1]

    p3 = np.zeros((100, P3_COLS), f)
    p3[0:100, 0:100] = np.asarray(inputs["W1"], f)
    p3[0:100, 100:200] = np.asarray(inputs["W2"], f)
    p3[0:100, 200:328] = np.asarray(inputs["W3"], f)
    return p1, p2, p3


def _numpy_reference(theta, W0, b0, W1, b1, W2, b2, W3, b3, knots, grid):
    lo = np.array([THETA_LO[0], THETA_LO[1]])
    sc = np.array([THETA_SCALE[0], THETA_SCALE[1]])
    t = (theta.astype(np.float64) - lo) / sc
    h = np.maximum(t @ W0 + b0, 0.0)
    h = np.maximum(h @ W1 + b1, 0.0)
    h = np.maximum(h @ W2 + b2, 0.0)
    out = h @ W3 + b3
    y = out.reshape(128, -1)
    kn = knots.astype(np.float64)
    h_k = kn[1:] - kn[:-1]
    rhs = 6.0 * ((y[2:] - y[1:-1]) / h_k[1:, None] - (y[1:-1] - y[:-2]) / h_k[:-1, None])
    diag = 2.0 * (h_k[:-1] + h_k[1:])
    off = h_k[1:-1]
    A = np.diag(diag) + np.diag(off, 1) + np.diag(off, -1)
    M_inner = np.linalg.solve(A, rhs)
    M = np.concatenate([np.zeros((1, y.shape[1])), M_inner,
                        np.zeros((1, y.shape[1]))], axis=0)
    hk = h_k[:, None]
    a = y[:-1]
    b = (y[1:] - y[:-1]) / hk - hk * (2.0 * M[:-1] + M[1:]) / 6.0
    c = M[:-1] / 2.0
    d = (M[1:] - M[:-1]) / (6.0 * hk)
    x = grid.astype(np.float64).reshape(-1)
    idx = np.clip(np.searchsorted(kn, x, side='right') - 1, 0, 126)
    fr = (x - kn[idx])[:, None]
    val = a[idx] + fr * (b[idx] + fr * (c[idx] + fr * d[idx]))
    val = val.reshape(grid.shape[0], grid.shape[1], -1)
    return np.ascontiguousarray(np.moveaxis(val, -1, 0)).astype(np.float32)


def kernel(**inputs):
    if not _fast_path_ok(inputs):
        args = {k: np.asarray(v, dtype=np.float32) for k, v in inputs.items()}
        return _numpy_reference(**args)

    from concourse.bass_utils import run_bass_kernel_spmd

    if "nc" not in _CACHE:
        _CACHE["nc"] = _build_program()
    nc = _CACHE["nc"]

    grid = np.asarray(inputs["grid"], dtype=np.float32)
    blk = np.ascontiguousarray(grid[:129, :129]).reshape(-1)
    xpad = np.zeros(N_CORES * PTS, dtype=np.float32)
    xpad[:UNIQ] = blk
    p1, p2, p3 = _pack_inputs(inputs)
    in_maps = []
    for c in range(N_CORES):
        xc = xpad[c * PTS:(c + 1) * PTS].reshape(GRP, P)
        p2c = p2.copy()
        p2c[:, 4:268] = xc[np.arange(128) % 8]
        in_maps.append(dict(p1=p1, p2=np.ascontiguousarray(p2c), p3=p3))
    res = run_bass_kernel_spmd(nc, in_maps, list(range(N_CORES)),
                               trace=bool(_CACHE.get("trace", False)),
                               tmpdir=_CACHE.get("tmpdir"))
    _CACHE["last_res"] = res
    vals = np.concatenate(
        [np.asarray(res.results[c]["out"]).astype(np.float32)
         for c in range(N_CORES)], axis=1)[:, :UNIQ]
    vb = vals.reshape(256, 129, 129)
    full = np.empty((256, 256, 256), dtype=np.float32)
    full[:, :129, :129] = vb
    full[:, 129:, :129] = vb[:, 127:0:-1, :]
    full[:, :, 129:] = full[:, :, 127:0:-1]
    return full
